# revision 18
# baseline (speedup 1.0000x reference)
"""Multi-head attention kernel for 8 Trainium2 NeuronCores (no collective).

Problem: B=2, S=2048, H=8, DK=DV=64, D=512 (nn_MultiHeadAttention).

Sharding: core c owns batch b=c//4 and query rows [512*r, 512*r+512), r=c%4.
Every core computes ALL K'/V' projections locally from the full K/V (which it
must load anyway). All matmuls are f16: fp8 fails accuracy (relative error of
a random-sign dot product does not shrink with contraction length).

Changes vs the 135us baseline (measured ~130-134us, rel err 6.7e-4):
  - Batched input DMAs (one dma_start per weight tensor / qt / kt slab / vt
    group of 4 tiles) - the sync engine was serializing ~33 issues at ~600ns.
  - oA/oB merged into one 2-bank PSUM tile -> single den2/o65 copies.
  - Last pair's denominator broadcast moved from gpsimd (~3.3us chain) to a
    K=1 ones-matmul on the PE (~0.4us), keeping the PE warm through the tail,
    with an interleaved out-proj epilogue (no sc-ring circular waits).
  - Output ring bufs=4 and f16 output stores (host upcasts) so the final
    DMAs issue back-to-back instead of serializing on a 2-deep ring.
  - HAM warmup: dummy matmuls flip the PE clock gate to 2.4 GHz during the
    DMA-bound prologue and bridge inter-stage arrival gaps.

Measured floor of this design in this environment: ~6.7us framework preamble
+ ~20us DMA-bound prologue (~230 GB/s effective) + ~90us PE-bound main loop
+ ~9us tail + ~6.5us teardown. Cross-core K/V-projection dedup (remote_dma
works; collectives do not) is defeated by 1-4.5ms inter-core launch/upload
skew under the axon tunnel, so every core stays fully independent.

Per-core dataflow (heads processed as 4 pairs of 2; projections drip-fed
into the attention pipeline so the load DMAs overlap compute):
  QT[p]  = wq[p].T @ qT + bq          [128, 512] f16
  KT[p]  = wk[p].T @ kT + bk          [128, 2048] f16
  V'[t]  = vT[t].T @ wv + bv | 1      [128, 8, 65] f16 (ones col -> denom)
  scoresT= KT[p] tile @ QT[p]         2 heads packed via tile_position
  at     = exp(scoresT/8)             ACT -> f16, no max-subtract (overflow
                                      safe: scores ~ N(0,1))
  o65   += V'[t,h].T @ at[h]          accumulated over t; row 64 = denom
  o2T[p] = o65[0:64] * bcast(1/den)   denominator row -> broadcast -> DVE
                                      fast reciprocal
  out    = sum_p o2T[p].T @ wo2[p]    K=128 pair-packed matmuls + bo;
                                      pairs 0-2 accumulate during the last
                                      pair's normalization window
"""

import numpy as np

B, S, H, DK, DV = 2, 2048, 8, 64, 64
D = H * DV  # 512
NCORES = 8
ROWS = (B * S) // NCORES  # 512 query rows per core
NPAIR = H // 2  # 4 head pairs
NTT = S // 128  # 16 key tiles
NDC = D // 128  # 4 contraction chunks
NSLAB = 4  # key slabs of 512
SLAB = S // NSLAB
P = 128
VW = DV + 1  # 65: V columns per head incl. the ones column

_prog = {}


def _build_program():
    from contextlib import ExitStack

    import concourse.mybir as mybir
    import concourse.tile as tile
    from concourse import bacc

    f32 = mybir.dt.float32
    f16 = mybir.dt.float16
    Exp = mybir.ActivationFunctionType.Exp

    nc = bacc.Bacc("TRN2", target_bir_lowering=False, debug=False, num_devices=NCORES)

    c1_d = nc.dram_tensor("c1", [P, NDC, D + ROWS], f16, kind="ExternalInput").ap()
    c2_d = nc.dram_tensor("c2", [P, NDC, D + SLAB], f16, kind="ExternalInput").ap()
    kt_d = nc.dram_tensor("kt", [NSLAB - 1, P, NDC, SLAB], f16, kind="ExternalInput").ap()
    vt_d = nc.dram_tensor("vt", [4, P, 4, NDC, P], f16, kind="ExternalInput").ap()
    wv_d = nc.dram_tensor("wv", [P, NDC, D], f16, kind="ExternalInput").ap()
    wo_d = nc.dram_tensor("wo", [P, NPAIR, D], f16, kind="ExternalInput").ap()
    bqk_d = nc.dram_tensor("bqk", [P, 2, NPAIR], f32, kind="ExternalInput").ap()
    b2_d = nc.dram_tensor("b2", [1, 2, D], f32, kind="ExternalInput").ap()
    out_d = nc.dram_tensor("out", [ROWS // P, P, D], f16, kind="ExternalOutput").ap()

    with tile.TileContext(nc) as tc, ExitStack() as ctx:
        weights = ctx.enter_context(tc.tile_pool(name="weights", bufs=1))
        raw = ctx.enter_context(tc.tile_pool(name="raw", bufs=1))
        acts = ctx.enter_context(tc.tile_pool(name="acts", bufs=1))
        work = ctx.enter_context(tc.tile_pool(name="work", bufs=1))
        # PSUM: sc ring 3x2 banks (scores + all projection/outproj scratch +
        # last-pair denominator broadcast), merged o-accumulator 2 banks ->
        # exactly 8 banks.
        ps_sc = ctx.enter_context(tc.tile_pool(name="ps_sc", bufs=3, space="PSUM"))
        ps_o = ctx.enter_context(tc.tile_pool(name="ps_o", bufs=1, space="PSUM"))

        # ---------------- load phase ----------------
        c1_sb = raw.tile([P, NDC, D + ROWS], f16, tag="c1", name="c1")
        c2_sb = raw.tile([P, NDC, D + SLAB], f16, tag="c2", name="c2")
        wq_sb = c1_sb[:, :, 0:D]
        qt_sb = c1_sb[:, :, D : D + ROWS]
        wk_sb = c2_sb[:, :, 0:D]
        wv_sb = weights.tile([P, NDC, D], f16, tag="wv", name="wv")
        wo_sb = weights.tile([P, NPAIR, D], f16, tag="wo", name="wo")
        kt_rest = [
            raw.tile([P, NDC, SLAB], f16, tag=f"kt{g}", name=f"kt{g}")
            for g in range(1, NSLAB)
        ]
        kt_sb = [c2_sb[:, :, D : D + SLAB]] + kt_rest
        vt_sb = [
            raw.tile([P, 4, NDC, P], f16, tag=f"vt{gr}", name=f"vt{gr}")
            for gr in range(4)
        ]
        bqk_sb = weights.tile([P, 2, NPAIR], f32, tag="bqk")
        b2_sb = weights.tile([1, 2, D], f32, tag="b2")
        bvb_sb = weights.tile([P, D], f32, tag="bvb")
        bob_sb = weights.tile([P, D], f32, tag="bob")
        ones16 = weights.tile([1, DV], f16, tag="ones16")
        nc.gpsimd.memset(ones16, 1.0)
        # HAM warmup: ~12 dummy matmuls flip the PE clock gate to 2.4 GHz
        # (~3.4us of sustained PE activity) while the input DMAs are still in
        # flight, so the prologue projections run at full rate.
        zq = weights.tile([P, ROWS], f16, tag="zq")
        nc.vector.memset(zq, 0.0)
        # Load order = consumption order. One dma_start per tensor / slab /
        # vt-group on the sync HWDGE queue: fewer+bigger issues win, and the
        # scalar (ACT) / gpsimd (SWDGE) alternatives both measured slower.
        nc.sync.dma_start(out=c1_sb, in_=c1_d)
        nc.sync.dma_start(out=bqk_sb, in_=bqk_d)
        nc.sync.dma_start(out=c2_sb, in_=c2_d)
        nc.sync.dma_start(out=wv_sb, in_=wv_d)
        nc.sync.dma_start(out=b2_sb, in_=b2_d)
        nc.gpsimd.partition_broadcast(bvb_sb, b2_sb[:, 0, :], channels=P)
        nc.sync.dma_start(out=vt_sb[0], in_=vt_d[0])
        for g in range(1, 4):
            nc.sync.dma_start(out=kt_sb[g], in_=kt_d[g - 1])
            nc.sync.dma_start(out=vt_sb[g], in_=vt_d[g])
        nc.sync.dma_start(out=wo_sb, in_=wo_d)
        nc.gpsimd.partition_broadcast(bob_sb, b2_sb[:, 1, :], channels=P)

        # ---------------- persistent compute tiles ----------------
        KT = [acts.tile([P, S], f16, tag=f"KT{p}", name=f"KT{p}") for p in range(NPAIR)]
        QT = [acts.tile([P, ROWS], f16, tag=f"QT{p}", name=f"QT{p}") for p in range(NPAIR)]
        o2T = [acts.tile([P, ROWS], f16, tag=f"o2T{p}", name=f"o2T{p}") for p in range(NPAIR)]
        V16 = [acts.tile([P, H, VW], f16, tag=f"V16{t}", name=f"V16{t}") for t in range(NTT)]

        def sc_tile(name):
            return ps_sc.tile([P, 2, ROWS], f32, tag="sc", name=name)

        def proj_q(p):
            ps = sc_tile("ps_q")
            for c in range(NDC):
                nc.tensor.matmul(
                    ps[:, 0, :], lhsT=wq_sb[:, c, p * P : (p + 1) * P], rhs=qt_sb[:, c, :],
                    start=(c == 0), stop=(c == NDC - 1),
                )
            nc.vector.tensor_scalar_add(QT[p], ps[:, 0, :], bqk_sb[:, 0, p : p + 1])

        def proj_kt(p, g):
            ps = sc_tile("ps_k")
            for c in range(NDC):
                nc.tensor.matmul(
                    ps[:, 0, :],
                    lhsT=wk_sb[:, c, p * P : (p + 1) * P],
                    rhs=kt_sb[g][:, c, :],
                    start=(c == 0), stop=(c == NDC - 1),
                )
            nc.vector.tensor_scalar_add(
                KT[p][:, g * SLAB : (g + 1) * SLAB], ps[:, 0, :], bqk_sb[:, 1, p : p + 1]
            )

        def proj_v(t):
            ps = sc_tile("ps_v")
            for c in range(NDC):
                nc.tensor.matmul(
                    ps[:, 0, :], lhsT=vt_sb[t // 4][:, t % 4, c, :], rhs=wv_sb[:, c, :],
                    start=(c == 0), stop=(c == NDC - 1),
                )
            nc.vector.tensor_add(
                V16[t][:, :, 0:DV],
                ps[:, 0, :].rearrange("p (h v) -> p h v", h=H),
                bvb_sb.rearrange("p (h v) -> p h v", h=H),
            )
            nc.vector.memset(V16[t][:, :, DV:VW], 1.0)

        # ---------------- prologue ----------------
        # Initial warmup flips HAM while the first loads are in flight; the
        # small dummy bursts between stages bridge the DMA-arrival gaps so the
        # PE never sits idle past the 3.4us MID window and re-throttles.
        def warm(n):
            wps = sc_tile("ps_warm")
            for _ in range(n):
                nc.tensor.matmul(
                    wps[:, 0, :], lhsT=zq[:, 0:P], rhs=zq, start=True, stop=True
                )

        warm(12)
        proj_q(0)
        warm(6)
        proj_kt(0, 0)
        warm(6)
        proj_v(0)
        warm(6)
        proj_v(1)
        warm(6)

        # ---------------- pair pipeline ----------------
        out_part = []  # held output-projection accumulators (pairs 0-2)
        for p in range(NPAIR):
            oAB = ps_o.tile([VW, 2, ROWS], f32, tag="oab", name="oAB")
            for t in range(NTT):
                # drip-feed remaining projection work into the pair windows
                # (K slabs staged so their DMAs have time to land)
                if p == 0:
                    if t in (2, 6, 10):
                        proj_kt(0, t // 4 + 1)
                    elif t == 4:
                        proj_q(1)
                    if t < NTT - 2:
                        proj_v(t + 2)
                if p == 1 and t == 0:
                    proj_q(2)
                if p == 2 and t == 0:
                    proj_q(3)
                if p < NPAIR - 1 and 11 <= t < 15:
                    proj_kt(p + 1, t - 11)

                ts = slice(t * P, (t + 1) * P)
                ps = sc_tile("ps_sc")
                nc.tensor.matmul(
                    ps[:, 0, :], lhsT=KT[p][0:64, ts], rhs=QT[p][0:64, :],
                    start=True, stop=True, tile_position=(0, 0),
                )
                nc.tensor.matmul(
                    ps[:, 1, :], lhsT=KT[p][64:P, ts], rhs=QT[p][64:P, :],
                    start=True, stop=True, tile_position=(64, 0),
                )
                at = work.tile([P, 2, ROWS], f16, tag="at", name="at", bufs=6)
                nc.scalar.activation(at, ps, Exp, scale=1.0 / np.sqrt(DK))
                first, last = (t == 0), (t == NTT - 1)
                nc.tensor.matmul(
                    oAB[:, 0, :], lhsT=V16[t][:, 2 * p, :], rhs=at[:, 0, :],
                    start=first, stop=last,
                )
                nc.tensor.matmul(
                    oAB[:, 1, :], lhsT=V16[t][:, 2 * p + 1, :], rhs=at[:, 1, :],
                    start=first, stop=last,
                )

            # Normalization: denominator row (row 64) -> broadcast over 64
            # partitions -> fast approximate reciprocal -> multiplies into the
            # o2T halves. For pairs 0-2 the o65 accumulator is evicted to
            # SBUF first so the single ps_o ring frees quickly and the chain
            # (on gpsimd+DVE) overlaps the next pair; the last (tail-exposed)
            # pair instead broadcasts via a K=1 ones-matmul on the PE and
            # normalizes directly from PSUM, keeping the PE warm.
            if p < NPAIR - 1:
                den2 = work.tile([1, 2, ROWS], f32, tag="den2", name="den2", bufs=2)
                nc.vector.tensor_copy(den2, oAB[DV : DV + 1, :, :])
                o65 = work.tile([VW, 2, ROWS], f32, tag="o65", name="o65", bufs=2)
                nc.vector.tensor_copy(o65, oAB)
                denb = work.tile([64, 2, ROWS], f32, tag="denb", name="denb", bufs=2)
                nc.gpsimd.partition_broadcast(
                    denb.rearrange("p a b -> p (a b)"),
                    den2.rearrange("p a b -> p (a b)"),
                    channels=64,
                )
                rb = work.tile([64, 2, ROWS], f32, tag="rb", name="rb", bufs=2)
                nc.vector.reciprocal_approx_fast(rb, denb)
                nc.vector.tensor_mul(o2T[p][0:64, :], o65[0:DV, 0, :], rb[:, 0, :])
                nc.vector.tensor_mul(o2T[p][64:P, :], o65[0:DV, 1, :], rb[:, 1, :])
            else:
                den16 = work.tile([1, 2, ROWS], f16, tag="den16", name="den16")
                nc.vector.tensor_copy(den16, oAB[DV : DV + 1, :, :])
                # Broadcast the denominator row over 64 partitions with a K=1
                # ones-matmul on the PE (the gpsimd partition_broadcast chain
                # costs ~3us and lets the PE go cold through the tail).
                ps_b = sc_tile("ps_bcast")
                for hh in range(2):
                    nc.tensor.matmul(
                        ps_b[0:64, hh, :],
                        lhsT=ones16,
                        rhs=den16[:, hh, :],
                        start=True, stop=True,
                    )
                rb = work.tile([64, 2, ROWS], f32, tag="rb", name="rb", bufs=2)
                nc.vector.reciprocal_approx_fast(rb, ps_b[0:64, :, :])
                # Fill the norm-chain gap: output projection over pairs 0-2
                # runs on the otherwise-idle PE while the chain drains. The
                # epilogue is interleaved so each sc-ring slot is freed (by the
                # bias-add read) before the slot is needed again.
                def op_acc(st):
                    t_ = sc_tile(f"ps_out{st}")
                    out_part.append(t_)
                    for pp in range(NPAIR - 1):
                        nc.tensor.matmul(
                            t_[:, 0, :], lhsT=o2T[pp][:, st * P : (st + 1) * P],
                            rhs=wo_sb[:, pp, :], start=(pp == 0), stop=False,
                        )

                def op_fin(st):
                    nc.tensor.matmul(
                        out_part[st][:, 0, :],
                        lhsT=o2T[NPAIR - 1][:, st * P : (st + 1) * P],
                        rhs=wo_sb[:, NPAIR - 1, :],
                        start=False, stop=True,
                    )
                    ot = work.tile([P, D], f16, tag="ot", name="ot", bufs=4)
                    nc.vector.tensor_add(ot, out_part[st][:, 0, :], bob_sb)
                    nc.sync.dma_start(out=out_d[st], in_=ot)

                op_acc(0)
                op_acc(1)
                nc.vector.tensor_mul(o2T[p][0:64, :], oAB[0:DV, 0, :], rb[:, 0, :])
                nc.vector.tensor_mul(o2T[p][64:P, :], oAB[0:DV, 1, :], rb[:, 1, :])
                op_fin(0)
                op_acc(2)
                op_fin(1)
                op_acc(3)
                op_fin(2)
                op_fin(3)

    nc.compile()
    return nc


def _get_program(repeats=1, hw_loop=0):
    key = (repeats, hw_loop)
    if key not in _prog:
        _prog[key] = _build_program()
    return _prog[key]


def _stage_inputs(queries, keys, values, wq, bq, wk, bk, wv, bv, wo, bo):
    """Host staging: transpose activations to [D, S], chunk contractions with
    the partition dim leading, per-core query shards. Returns the 8 per-core
    input dicts."""
    h = np.float16

    qT = [np.ascontiguousarray(queries[b].T) for b in range(B)]
    kT = [np.ascontiguousarray(keys[b].T) for b in range(B)]
    vT = [np.ascontiguousarray(values[b].T) for b in range(B)]

    def chunkP(m):  # [512, N] -> [128, 4, N] f16 (partition-major)
        return np.ascontiguousarray(
            m.reshape(NDC, P, m.shape[1]).transpose(1, 0, 2)
        ).astype(h)

    wq_m = chunkP(np.concatenate([wq[i] for i in range(H)], axis=1))
    wk_m = chunkP(np.concatenate([wk[i] for i in range(H)], axis=1))
    wv_m = chunkP(np.concatenate([wv[i] for i in range(H)], axis=1))
    wo2 = np.ascontiguousarray(
        wo.reshape(NPAIR, P, D).transpose(1, 0, 2)
    ).astype(h)  # [128, 4, 512]
    bqk = np.empty((P, 2, NPAIR), np.float32)
    bqk[:, 0, :] = bq.reshape(NPAIR, P).T
    bqk[:, 1, :] = bk.reshape(NPAIR, P).T
    b2 = np.stack([bv.reshape(D), bo.reshape(D)])[None].astype(np.float32)  # [1, 2, 512]

    # kt[g, kappa, c, j] = kT[c*128 + kappa, g*512 + j]
    kt_b = [
        np.ascontiguousarray(
            kT[b].reshape(NDC, P, NSLAB, SLAB).transpose(2, 1, 0, 3)
        ).astype(h)
        for b in range(B)
    ]
    # vt[gr, kappa, ti, c, j] = vT[c*128 + kappa, (gr*4 + ti)*128 + j]
    vt_b = [
        np.ascontiguousarray(
            vT[b].reshape(NDC, P, 4, 4, P).transpose(2, 1, 3, 0, 4)
        ).astype(h)
        for b in range(B)
    ]

    c2_b = [
        np.ascontiguousarray(np.concatenate([wk_m, kt_b[b][0]], axis=2))
        for b in range(B)
    ]
    in_maps = []
    for c in range(NCORES):
        b, r = c // 4, c % 4
        qt_c = chunkP(qT[b][:, r * ROWS : (r + 1) * ROWS])
        c1 = np.ascontiguousarray(np.concatenate([wq_m, qt_c], axis=2))
        in_maps.append(
            {
                "c1": c1, "c2": c2_b[b], "kt": kt_b[b][1:], "vt": vt_b[b],
                "wv": wv_m, "wo": wo2, "bqk": bqk, "b2": b2,
            }
        )
    return in_maps


def run(trace=False, repeats=1, hw_loop=0, **inputs):
    """Run the kernel; returns (output, BassKernelResults)."""
    from concourse.bass_utils import run_bass_kernel_spmd

    nc = _get_program(repeats, hw_loop)
    in_maps = _stage_inputs(**inputs)
    res = run_bass_kernel_spmd(nc, in_maps, core_ids=list(range(NCORES)), trace=trace)
    out = np.empty((B, S, D), np.float32)
    for c in range(NCORES):
        b, r = c // 4, c % 4
        out[b, r * ROWS : (r + 1) * ROWS, :] = res.results[c]["out"].reshape(ROWS, D)
    return out, res


def kernel(**inputs):
    out, _ = run(trace=False, **inputs)
    return out


# revision 21
# speedup vs baseline: 1.0223x; 1.0223x over previous
"""Multi-head attention kernel for 8 Trainium2 NeuronCores (no collective).

Problem: B=2, S=2048, H=8, DK=DV=64, D=512 (nn_MultiHeadAttention).

Sharding: core c owns batch b=c//4 and query rows [512*r, 512*r+512), r=c%4.
Every core computes ALL K'/V' projections locally from the full K/V (which it
must load anyway). All matmuls are f16: fp8 fails accuracy (relative error of
a random-sign dot product does not shrink with contraction length).

Changes vs the 135us baseline (measured ~130-134us, rel err 6.7e-4):
  - Batched input DMAs (one dma_start per weight tensor / qt / kt slab / vt
    group of 4 tiles) - the sync engine was serializing ~33 issues at ~600ns.
  - oA/oB merged into one 2-bank PSUM tile -> single den2/o65 copies.
  - Last pair's denominator broadcast moved from gpsimd (~3.3us chain) to a
    K=1 ones-matmul on the PE (~0.4us), keeping the PE warm through the tail,
    with an interleaved out-proj epilogue (no sc-ring circular waits).
  - Output ring bufs=4 and f16 output stores (host upcasts) so the final
    DMAs issue back-to-back instead of serializing on a 2-deep ring.
  - HAM warmup: dummy matmuls flip the PE clock gate to 2.4 GHz during the
    DMA-bound prologue and bridge inter-stage arrival gaps.

Measured floor of this design in this environment: ~6.7us framework preamble
+ ~20us DMA-bound prologue (~230 GB/s effective) + ~90us PE-bound main loop
+ ~9us tail + ~6.5us teardown. Cross-core K/V-projection dedup (remote_dma
works; collectives do not) is defeated by 1-4.5ms inter-core launch/upload
skew under the axon tunnel, so every core stays fully independent.

Per-core dataflow (heads processed as 4 pairs of 2; projections drip-fed
into the attention pipeline so the load DMAs overlap compute):
  QT[p]  = wq[p].T @ qT + bq          [128, 512] f16
  KT[p]  = wk[p].T @ kT + bk          [128, 2048] f16
  V'[t]  = vT[t].T @ wv + bv | 1      [128, 8, 65] f16 (ones col -> denom)
  scoresT= KT[p] tile @ QT[p]         2 heads packed via tile_position
  at     = exp(scoresT/8)             ACT -> f16, no max-subtract (overflow
                                      safe: scores ~ N(0,1))
  o65   += V'[t,h].T @ at[h]          accumulated over t; row 64 = denom
  o2T[p] = o65[0:64] * bcast(1/den)   denominator row -> broadcast -> DVE
                                      fast reciprocal
  out    = sum_p o2T[p].T @ wo2[p]    K=128 pair-packed matmuls + bo;
                                      pairs 0-2 accumulate during the last
                                      pair's normalization window
"""

import numpy as np

B, S, H, DK, DV = 2, 2048, 8, 64, 64
D = H * DV  # 512
NCORES = 8
ROWS = (B * S) // NCORES  # 512 query rows per core
NPAIR = H // 2  # 4 head pairs
NTT = S // 128  # 16 key tiles
NDC = D // 128  # 4 contraction chunks
NSLAB = 4  # key slabs of 512
SLAB = S // NSLAB
P = 128
VW = DV + 1  # 65: V columns per head incl. the ones column

_prog = {}


def _build_program():
    from contextlib import ExitStack

    import concourse.mybir as mybir
    import concourse.tile as tile
    from concourse import bacc

    f32 = mybir.dt.float32
    f16 = mybir.dt.float16
    Exp = mybir.ActivationFunctionType.Exp

    nc = bacc.Bacc("TRN2", target_bir_lowering=False, debug=False, num_devices=NCORES)

    qt_d = nc.dram_tensor("qt", [P, NDC, ROWS], f16, kind="ExternalInput").ap()
    kt_d = nc.dram_tensor("kt", [NSLAB, P, NDC, SLAB], f16, kind="ExternalInput").ap()
    vt_d = nc.dram_tensor("vt", [4, P, 4, NDC, P], f16, kind="ExternalInput").ap()
    wq_d = nc.dram_tensor("wq", [P, NDC, D], f16, kind="ExternalInput").ap()
    wk_d = nc.dram_tensor("wk", [P, NDC, D], f16, kind="ExternalInput").ap()
    wv_d = nc.dram_tensor("wv", [P, NDC, D], f16, kind="ExternalInput").ap()
    wo_d = nc.dram_tensor("wo", [P, NPAIR, D], f16, kind="ExternalInput").ap()
    bqk_d = nc.dram_tensor("bqk", [P, 2, NPAIR], f32, kind="ExternalInput").ap()
    b2_d = nc.dram_tensor("b2", [1, 2, D], f32, kind="ExternalInput").ap()
    out_d = nc.dram_tensor("out", [ROWS // P, P, D], f16, kind="ExternalOutput").ap()

    with tile.TileContext(nc) as tc, ExitStack() as ctx:
        weights = ctx.enter_context(tc.tile_pool(name="weights", bufs=1))
        raw = ctx.enter_context(tc.tile_pool(name="raw", bufs=1))
        acts = ctx.enter_context(tc.tile_pool(name="acts", bufs=1))
        work = ctx.enter_context(tc.tile_pool(name="work", bufs=1))
        # PSUM: sc ring 3x2 banks (scores + all projection/outproj scratch +
        # last-pair denominator broadcast), merged o-accumulator 2 banks ->
        # exactly 8 banks.
        ps_sc = ctx.enter_context(tc.tile_pool(name="ps_sc", bufs=3, space="PSUM"))
        ps_o = ctx.enter_context(tc.tile_pool(name="ps_o", bufs=1, space="PSUM"))

        # ---------------- load phase ----------------
        wq_sb = weights.tile([P, NDC, D], f16, tag="wq", name="wq")
        wk_sb = weights.tile([P, NDC, D], f16, tag="wk", name="wk")
        wv_sb = weights.tile([P, NDC, D], f16, tag="wv", name="wv")
        wo_sb = weights.tile([P, NPAIR, D], f16, tag="wo", name="wo")
        qt_sb = raw.tile([P, NDC, ROWS], f16, tag="qt", name="qt")
        kt_sb = [
            raw.tile([P, NDC, SLAB], f16, tag=f"kt{g}", name=f"kt{g}")
            for g in range(NSLAB)
        ]
        vt_sb = [
            raw.tile([P, 4, NDC, P], f16, tag=f"vt{gr}", name=f"vt{gr}")
            for gr in range(4)
        ]
        bqk_sb = weights.tile([P, 2, NPAIR], f32, tag="bqk")
        b2_sb = weights.tile([1, 2, D], f32, tag="b2")
        bvb_sb = weights.tile([P, D], f32, tag="bvb")
        bob_sb = weights.tile([P, D], f32, tag="bob")
        ones16 = weights.tile([1, DV], f16, tag="ones16")
        nc.gpsimd.memset(ones16, 1.0)
        # HAM warmup: ~12 dummy matmuls flip the PE clock gate to 2.4 GHz
        # (~3.4us of sustained PE activity) while the input DMAs are still in
        # flight, so the prologue projections run at full rate.
        zq = weights.tile([P, ROWS], f16, tag="zq")
        nc.vector.memset(zq, 0.0)
        # Load order = consumption order. One dma_start per tensor / slab /
        # vt-group on the sync HWDGE queue: fewer+bigger issues win, and the
        # scalar (ACT) / gpsimd (SWDGE) alternatives both measured slower.
        nc.sync.dma_start(out=wq_sb, in_=wq_d)
        nc.sync.dma_start(out=qt_sb, in_=qt_d)
        nc.sync.dma_start(out=bqk_sb, in_=bqk_d)
        nc.sync.dma_start(out=wk_sb, in_=wk_d)
        nc.sync.dma_start(out=kt_sb[0], in_=kt_d[0])
        nc.sync.dma_start(out=wv_sb, in_=wv_d)
        nc.sync.dma_start(out=b2_sb, in_=b2_d)
        nc.gpsimd.partition_broadcast(bvb_sb, b2_sb[:, 0, :], channels=P)
        nc.sync.dma_start(out=vt_sb[0], in_=vt_d[0])
        for g in range(1, 4):
            nc.sync.dma_start(out=kt_sb[g], in_=kt_d[g])
            nc.sync.dma_start(out=vt_sb[g], in_=vt_d[g])
        nc.sync.dma_start(out=wo_sb, in_=wo_d)
        nc.gpsimd.partition_broadcast(bob_sb, b2_sb[:, 1, :], channels=P)

        # ---------------- persistent compute tiles ----------------
        KT = [acts.tile([P, S], f16, tag=f"KT{p}", name=f"KT{p}") for p in range(NPAIR)]
        QT = [acts.tile([P, ROWS], f16, tag=f"QT{p}", name=f"QT{p}") for p in range(NPAIR)]
        o2T = [acts.tile([P, ROWS], f16, tag=f"o2T{p}", name=f"o2T{p}") for p in range(NPAIR)]
        V16 = [acts.tile([P, H, VW], f16, tag=f"V16{t}", name=f"V16{t}") for t in range(NTT)]

        def sc_tile(name):
            return ps_sc.tile([P, 2, ROWS], f32, tag="sc", name=name)

        def proj_q(p):
            ps = sc_tile("ps_q")
            for c in range(NDC):
                nc.tensor.matmul(
                    ps[:, 0, :], lhsT=wq_sb[:, c, p * P : (p + 1) * P], rhs=qt_sb[:, c, :],
                    start=(c == 0), stop=(c == NDC - 1),
                )
            nc.vector.tensor_scalar_add(QT[p], ps[:, 0, :], bqk_sb[:, 0, p : p + 1])

        def proj_kt(p, g):
            ps = sc_tile("ps_k")
            for c in range(NDC):
                nc.tensor.matmul(
                    ps[:, 0, :],
                    lhsT=wk_sb[:, c, p * P : (p + 1) * P],
                    rhs=kt_sb[g][:, c, :],
                    start=(c == 0), stop=(c == NDC - 1),
                )
            nc.vector.tensor_scalar_add(
                KT[p][:, g * SLAB : (g + 1) * SLAB], ps[:, 0, :], bqk_sb[:, 1, p : p + 1]
            )

        def proj_v(t):
            ps = sc_tile("ps_v")
            for c in range(NDC):
                nc.tensor.matmul(
                    ps[:, 0, :], lhsT=vt_sb[t // 4][:, t % 4, c, :], rhs=wv_sb[:, c, :],
                    start=(c == 0), stop=(c == NDC - 1),
                )
            nc.vector.tensor_add(
                V16[t][:, :, 0:DV],
                ps[:, 0, :].rearrange("p (h v) -> p h v", h=H),
                bvb_sb.rearrange("p (h v) -> p h v", h=H),
            )
            nc.vector.memset(V16[t][:, :, DV:VW], 1.0)

        # ---------------- prologue ----------------
        # Initial warmup flips HAM while the first loads are in flight; the
        # small dummy bursts between stages bridge the DMA-arrival gaps so the
        # PE never sits idle past the 3.4us MID window and re-throttles.
        def warm(n):
            wps = sc_tile("ps_warm")
            for _ in range(n):
                nc.tensor.matmul(
                    wps[:, 0, :], lhsT=zq[:, 0:P], rhs=zq, start=True, stop=True
                )

        warm(12)
        proj_q(0)
        warm(6)
        proj_kt(0, 0)
        warm(6)
        proj_v(0)
        warm(6)
        proj_v(1)
        warm(6)

        # ---------------- pair pipeline ----------------
        out_part = []  # held output-projection accumulators (pairs 0-2)
        for p in range(NPAIR):
            oAB = ps_o.tile([VW, 2, ROWS], f32, tag="oab", name="oAB")
            for t in range(NTT):
                # drip-feed remaining projection work into the pair windows
                # (K slabs staged so their DMAs have time to land)
                if p == 0:
                    if t in (2, 6, 10):
                        proj_kt(0, t // 4 + 1)
                    elif t == 4:
                        proj_q(1)
                    if t < NTT - 2:
                        proj_v(t + 2)
                if p == 1 and t == 0:
                    proj_q(2)
                if p == 2 and t == 0:
                    proj_q(3)
                if p < NPAIR - 1 and 11 <= t < 15:
                    proj_kt(p + 1, t - 11)

                ts = slice(t * P, (t + 1) * P)
                ps = sc_tile("ps_sc")
                nc.tensor.matmul(
                    ps[:, 0, :], lhsT=KT[p][0:64, ts], rhs=QT[p][0:64, :],
                    start=True, stop=True, tile_position=(0, 0),
                )
                nc.tensor.matmul(
                    ps[:, 1, :], lhsT=KT[p][64:P, ts], rhs=QT[p][64:P, :],
                    start=True, stop=True, tile_position=(64, 0),
                )
                at = work.tile([P, 2, ROWS], f16, tag="at", name="at", bufs=6)
                nc.scalar.activation(at, ps, Exp, scale=1.0 / np.sqrt(DK))
                first, last = (t == 0), (t == NTT - 1)
                nc.tensor.matmul(
                    oAB[:, 0, :], lhsT=V16[t][:, 2 * p, :], rhs=at[:, 0, :],
                    start=first, stop=last,
                )
                nc.tensor.matmul(
                    oAB[:, 1, :], lhsT=V16[t][:, 2 * p + 1, :], rhs=at[:, 1, :],
                    start=first, stop=last,
                )

            # Normalization: denominator row (row 64) -> broadcast over 64
            # partitions -> fast approximate reciprocal -> multiplies into the
            # o2T halves. For pairs 0-2 the o65 accumulator is evicted to
            # SBUF first so the single ps_o ring frees quickly and the chain
            # (on gpsimd+DVE) overlaps the next pair; the last (tail-exposed)
            # pair instead broadcasts via a K=1 ones-matmul on the PE and
            # normalizes directly from PSUM, keeping the PE warm.
            if p < NPAIR - 1:
                den2 = work.tile([1, 2, ROWS], f32, tag="den2", name="den2", bufs=2)
                nc.vector.tensor_copy(den2, oAB[DV : DV + 1, :, :])
                o65 = work.tile([VW, 2, ROWS], f32, tag="o65", name="o65", bufs=2)
                nc.vector.tensor_copy(o65, oAB)
                denb = work.tile([64, 2, ROWS], f32, tag="denb", name="denb", bufs=2)
                nc.gpsimd.partition_broadcast(
                    denb.rearrange("p a b -> p (a b)"),
                    den2.rearrange("p a b -> p (a b)"),
                    channels=64,
                )
                rb = work.tile([64, 2, ROWS], f32, tag="rb", name="rb", bufs=2)
                nc.vector.reciprocal_approx_fast(rb, denb)
                nc.vector.tensor_mul(o2T[p][0:64, :], o65[0:DV, 0, :], rb[:, 0, :])
                nc.vector.tensor_mul(o2T[p][64:P, :], o65[0:DV, 1, :], rb[:, 1, :])
            else:
                den16 = work.tile([1, 2, ROWS], f16, tag="den16", name="den16")
                nc.vector.tensor_copy(den16, oAB[DV : DV + 1, :, :])
                # Broadcast the denominator row over 64 partitions with a K=1
                # ones-matmul on the PE (the gpsimd partition_broadcast chain
                # costs ~3us and lets the PE go cold through the tail).
                ps_b = sc_tile("ps_bcast")
                for hh in range(2):
                    nc.tensor.matmul(
                        ps_b[0:64, hh, :],
                        lhsT=ones16,
                        rhs=den16[:, hh, :],
                        start=True, stop=True,
                    )
                rb = work.tile([64, 2, ROWS], f32, tag="rb", name="rb", bufs=2)
                nc.vector.reciprocal_approx_fast(rb, ps_b[0:64, :, :])
                # Fill the norm-chain gap: output projection over pairs 0-2
                # runs on the otherwise-idle PE while the chain drains. The
                # epilogue is interleaved so each sc-ring slot is freed (by the
                # bias-add read) before the slot is needed again.
                def op_acc(st):
                    t_ = sc_tile(f"ps_out{st}")
                    out_part.append(t_)
                    for pp in range(NPAIR - 1):
                        nc.tensor.matmul(
                            t_[:, 0, :], lhsT=o2T[pp][:, st * P : (st + 1) * P],
                            rhs=wo_sb[:, pp, :], start=(pp == 0), stop=False,
                        )

                def op_fin(st):
                    nc.tensor.matmul(
                        out_part[st][:, 0, :],
                        lhsT=o2T[NPAIR - 1][:, st * P : (st + 1) * P],
                        rhs=wo_sb[:, NPAIR - 1, :],
                        start=False, stop=True,
                    )
                    ot = work.tile([P, D], f16, tag="ot", name="ot", bufs=4)
                    nc.vector.tensor_add(ot, out_part[st][:, 0, :], bob_sb)
                    nc.sync.dma_start(out=out_d[st], in_=ot)

                op_acc(0)
                op_acc(1)
                nc.vector.tensor_mul(o2T[p][0:64, :], oAB[0:DV, 0, :], rb[:, 0, :])
                nc.vector.tensor_mul(o2T[p][64:P, :], oAB[0:DV, 1, :], rb[:, 1, :])
                op_fin(0)
                op_acc(2)
                op_fin(1)
                op_acc(3)
                op_fin(2)
                op_fin(3)

    nc.compile()
    return nc


def _get_program(repeats=1, hw_loop=0):
    key = (repeats, hw_loop)
    if key not in _prog:
        _prog[key] = _build_program()
    return _prog[key]


def _stage_inputs(queries, keys, values, wq, bq, wk, bk, wv, bv, wo, bo):
    """Host staging: transpose activations to [D, S], chunk contractions with
    the partition dim leading, per-core query shards. Returns the 8 per-core
    input dicts."""
    h = np.float16

    qT = [np.ascontiguousarray(queries[b].T) for b in range(B)]
    kT = [np.ascontiguousarray(keys[b].T) for b in range(B)]
    vT = [np.ascontiguousarray(values[b].T) for b in range(B)]

    def chunkP(m):  # [512, N] -> [128, 4, N] f16 (partition-major)
        return np.ascontiguousarray(
            m.reshape(NDC, P, m.shape[1]).transpose(1, 0, 2)
        ).astype(h)

    wq_m = chunkP(np.concatenate([wq[i] for i in range(H)], axis=1))
    wk_m = chunkP(np.concatenate([wk[i] for i in range(H)], axis=1))
    wv_m = chunkP(np.concatenate([wv[i] for i in range(H)], axis=1))
    wo2 = np.ascontiguousarray(
        wo.reshape(NPAIR, P, D).transpose(1, 0, 2)
    ).astype(h)  # [128, 4, 512]
    bqk = np.empty((P, 2, NPAIR), np.float32)
    bqk[:, 0, :] = bq.reshape(NPAIR, P).T
    bqk[:, 1, :] = bk.reshape(NPAIR, P).T
    b2 = np.stack([bv.reshape(D), bo.reshape(D)])[None].astype(np.float32)  # [1, 2, 512]

    # kt[g, kappa, c, j] = kT[c*128 + kappa, g*512 + j]
    kt_b = [
        np.ascontiguousarray(
            kT[b].reshape(NDC, P, NSLAB, SLAB).transpose(2, 1, 0, 3)
        ).astype(h)
        for b in range(B)
    ]
    # vt[gr, kappa, ti, c, j] = vT[c*128 + kappa, (gr*4 + ti)*128 + j]
    vt_b = [
        np.ascontiguousarray(
            vT[b].reshape(NDC, P, 4, 4, P).transpose(2, 1, 3, 0, 4)
        ).astype(h)
        for b in range(B)
    ]

    in_maps = []
    for c in range(NCORES):
        b, r = c // 4, c % 4
        qt_c = chunkP(qT[b][:, r * ROWS : (r + 1) * ROWS])
        in_maps.append(
            {
                "qt": qt_c, "kt": kt_b[b], "vt": vt_b[b],
                "wq": wq_m, "wk": wk_m, "wv": wv_m, "wo": wo2,
                "bqk": bqk, "b2": b2,
            }
        )
    return in_maps


def run(trace=False, repeats=1, hw_loop=0, **inputs):
    """Run the kernel; returns (output, BassKernelResults)."""
    from concourse.bass_utils import run_bass_kernel_spmd

    nc = _get_program(repeats, hw_loop)
    in_maps = _stage_inputs(**inputs)
    res = run_bass_kernel_spmd(nc, in_maps, core_ids=list(range(NCORES)), trace=trace)
    out = np.empty((B, S, D), np.float32)
    for c in range(NCORES):
        b, r = c // 4, c % 4
        out[b, r * ROWS : (r + 1) * ROWS, :] = res.results[c]["out"].reshape(ROWS, D)
    return out, res


def kernel(**inputs):
    out, _ = run(trace=False, **inputs)
    return out
